# revision 52
# baseline (speedup 1.0000x reference)
"""GAT-style message passing kernel for Trainium2 (8 NeuronCores, data-parallel over batch).

Reference math (per sample, 2 layers, shared weights):
    hidden = x @ W_in + b_in                      # [N, H]
    per layer:
        xt  = hidden @ W_t + b_t
        s_j = xt @ a_j ; s_i = xt @ a_i           # xt only feeds the scores
        att = softmax_j(lrelu(s_i[i] + s_j[j]))
        hidden = att @ hidden + hidden

Restructurings:
 1) Rank-23 factorization: hidden == U @ V with V = [W_in; b_in] constant and
    U0 = [x | 1 | s0_j | s0_i]; per layer U <- diag(1/D) (E^T U) + U.
    The two score columns ride along in U (s = U @ (V W_t a) is linear in U),
    so next-layer scores AND the softmax denominator (ones column) come out of
    the same E^T U product. V is applied once at the end.
 2) exp(lrelu(z)-C_i) = max(p_j, p'_j * g_i): one fused DVE tensor_scalar
    (mult+max) per E tile, fp16, no N^2 exp.
 3) E-stationary sweep: matmul(lhsT=E_chunk[128j,128i], rhs=U[128j,23])
    accumulates Y[i,0:23] in PSUM in NATURAL layout (i-block b row p is node
    16p+b), so the layer update needs no transposes at all.
 4) DVE does almost nothing but produce E tiles; the U update runs as two
    GPSIMD tensor_tensor ops after a PE mask-matmul broadcasts 1/D along the
    23 U columns; one E tile per sweep (the last-consumed chunk) also goes to
    GPSIMD.
 5) The last sample's final layer is processed group-major (4 i-blocks at a
    time) so eviction, U-update, and the hidden=U@V tail pipeline with E
    production instead of serializing after the sweep.
"""

import numpy as np
from contextlib import ExitStack

S = 2          # samples per core
N = 2048
Din = 20
UD = 23        # U columns: 20 x-features + ones + s_j + s_i
H = 128
NCH = 16       # chunks of 128 nodes (node n = 16p + c)
NUM_LAYERS = 2
N_CORES = 8
POOL_CS = {(0, 0): (13, 14, 15), (1, 0): (), (0, 1): ()}   # per-sweep GPSIMD E-chunks


def _build(ctx, tc, aps, ctot):
    import concourse.bass as bass
    from concourse import mybir

    nc = tc.nc
    f32 = mybir.dt.float32
    f16 = mybir.dt.float16
    Alu = mybir.AluOpType
    Act = mybir.ActivationFunctionType

    (g0bc_ap, p0_ap, u016_ap, u0f_ap, sel_ap, identf_ap, ident16_ap,
     v32_ap, cmask_ap, out_ap) = aps

    consts = ctx.enter_context(tc.tile_pool(name="consts", bufs=1))
    upool = ctx.enter_context(tc.tile_pool(name="upool", bufs=4))
    u16pool = ctx.enter_context(tc.tile_pool(name="u16pool", bufs=4))
    gpool = ctx.enter_context(tc.tile_pool(name="gpool", bufs=3))
    epool = ctx.enter_context(tc.tile_pool(name="epool", bufs=10))
    e4pool = ctx.enter_context(tc.tile_pool(name="e4pool", bufs=36))
    rbpool = ctx.enter_context(tc.tile_pool(name="rbpool", bufs=2))
    ypool = ctx.enter_context(tc.tile_pool(name="ypool", bufs=3))
    small = ctx.enter_context(tc.tile_pool(name="small", bufs=14))
    u2tp = ctx.enter_context(tc.tile_pool(name="u2tp", bufs=2))
    houtp = ctx.enter_context(tc.tile_pool(name="houtp", bufs=3))
    psY = ctx.enter_context(tc.tile_pool(name="psY", bufs=2, space="PSUM"))
    psA = ctx.enter_context(tc.tile_pool(name="psA", bufs=1, space="PSUM"))  # [128,2048] f32 = 4 banks
    psT = ctx.enter_context(tc.tile_pool(name="psT", bufs=1, space="PSUM"))
    psH = ctx.enter_context(tc.tile_pool(name="psH", bufs=1, space="PSUM"))

    # ---- input DMAs (queue order = first-consumption order) ----
    gbcs, p0s, u16s, ufs = [], [], [], []
    for s in range(S):
        gbc0 = gpool.tile([128, N], f16, tag="gbc", name=f"gbc0_{s}")
        nc.sync.dma_start(out=gbc0, in_=g0bc_ap[s])
        gbcs.append(gbc0)
        p0 = small.tile([128, NCH, 2], f32, tag="p0", name=f"p0_{s}")
        nc.sync.dma_start(out=p0, in_=p0_ap[s])
        p0s.append(p0)
        u016 = u16pool.tile([128, NCH, UD], f16, tag="u16", name=f"u016_{s}")
        nc.sync.dma_start(out=u016, in_=u016_ap[s])
        u16s.append(u016)
    for s in range(S):
        u0f = upool.tile([128, NCH, UD], f32, tag="un", name=f"u0f_{s}")
        nc.sync.dma_start(out=u0f, in_=u0f_ap[s])
        ufs.append(u0f)
    identf = consts.tile([128, 128], f32)
    nc.sync.dma_start(out=identf, in_=identf_ap)
    sel16 = consts.tile([NCH, NCH, 128], f16)
    nc.sync.dma_start(out=sel16, in_=sel_ap)
    cmask16 = consts.tile([NCH, 2, NCH * UD], f16)
    nc.sync.dma_start(out=cmask16, in_=cmask_ap)
    ident16 = consts.tile([128, 128], f16)
    nc.sync.dma_start(out=ident16, in_=ident16_ap)
    v92s = consts.tile([92, 4, 128], f16)
    nc.sync.dma_start(out=v92s, in_=v32_ap)
    ones_r = consts.tile([1, 128], f32)
    nc.vector.memset(ones_r, 1.0)

    un = {s: ufs[s] for s in range(S)}
    un16 = {s: u16s[s] for s in range(S)}
    # prep[s] = (gbc_tile, p_scalar_fn, pp_scalar_fn)
    prep = {s: (gbcs[s], (lambda s=s: (lambda c: p0s[s][:, c, 0:1]))(),
                (lambda s=s: (lambda c: p0s[s][:, c, 1:2]))()) for s in range(S)}

    def emit_prep(s, snew=None):
        """Scores -> p, p', gbc for the next layer of sample s. Reads scores
        from the early s-column update (snew) when given, so the chain to gbc
        does not wait for the full-U update."""
        if snew is None:
            u = un[s]
            sj, si = u[:, :, Din + 1], u[:, :, Din + 2]
        else:
            sj, si = snew[:, :, 0], snew[:, :, 1]
        mx = small.tile([1, 1], f32, tag="mx")
        nc.gpsimd.tensor_reduce(mx, sj, axis=mybir.AxisListType.XYZWC, op=Alu.max)
        psmb = psT.tile([128, 1], f32, tag="tp", name="psmb")
        nc.tensor.matmul(psmb, lhsT=ones_r, rhs=mx, start=True, stop=True)
        maxbc = small.tile([128, 1], f32, tag="maxbc")
        nc.scalar.copy(maxbc, psmb)
        negmax = small.tile([128, 1], f32, tag="negmax")
        nc.gpsimd.tensor_scalar(negmax, maxbc, -1.0, None, Alu.mult)
        negmax001 = small.tile([128, 1], f32, tag="negmax001")
        nc.gpsimd.tensor_scalar(negmax001, maxbc, -0.01, None, Alu.mult)
        p_sb = small.tile([128, NCH], f32, tag="p_sb")
        nc.scalar.activation(p_sb, sj, Act.Exp, bias=negmax[:, 0:1], scale=1.0)
        pp_sb = small.tile([128, NCH], f32, tag="pp_sb")
        nc.scalar.activation(pp_sb, sj, Act.Exp, bias=negmax001[:, 0:1], scale=0.01)
        u1 = small.tile([128, NCH], f32, tag="u1")
        nc.gpsimd.tensor_scalar(u1, si, maxbc[:, 0:1], float(ctot), Alu.add, Alu.add)
        u_sb = small.tile([128, NCH], f32, tag="u_sb")
        nc.gpsimd.tensor_scalar(u_sb, u1, -0.99, 10.5, Alu.mult, Alu.min)
        psuT = psT.tile([NCH, 128], f32, tag="tp", name="psuT")
        nc.tensor.transpose(psuT, u_sb, identf)
        g16 = small.tile([NCH, 128], f16, tag="g16")
        nc.scalar.activation(g16, psuT, Act.Exp)
        gbc = gpool.tile([128, N], f16, tag="gbc")
        psa = psA.tile([128, N], f32, tag="psa")
        for c in range(NCH):
            nc.tensor.matmul(psa[:, c * 128:(c + 1) * 128],
                             lhsT=sel16[:, c, :], rhs=g16,
                             start=True, stop=True)
        nc.scalar.copy(gbc, psa)
        prep[s] = (gbc, lambda c: p_sb[:, c:c + 1], lambda c: pp_sb[:, c:c + 1])

    def emit_sweep_E(s, L, mid=None, mid2=None):
        """E tiles for sample s layer L; Pool chunks first, then DVE chunks.
        `mid`/`mid2` are emitted after the 1st/8th DVE tile so the next
        transition's ops get early static-scheduler priority without idling
        DVE between sweeps."""
        gbc, pf, ppf = prep[s]
        pool_cs = POOL_CS[(s, L)]
        etiles = {}
        order = list(pool_cs) + [c for c in range(NCH) if c not in pool_cs]
        ndve = 0
        for c in order:
            eng = nc.gpsimd if c in pool_cs else nc.vector
            e_t = epool.tile([128, N], f16, tag="e", name=f"e{c}")
            eng.tensor_scalar(e_t, gbc, ppf(c), pf(c), Alu.mult, Alu.max)
            etiles[c] = e_t
            if eng is nc.vector:
                ndve += 1
                if ndve == 1 and mid is not None:
                    mid()
                if ndve == 8 and mid2 is not None:
                    mid2()
        return etiles

    def emit_sweep_MM(s, etiles):
        """E-stationary accumulation: Y[i,0:23] lands in natural layout."""
        u16 = un16[s]
        psy = psY.tile([128, 512], f32, tag="psy")
        for c in range(NCH):
            for b in range(NCH):
                nc.tensor.matmul(psy[:, UD * b:UD * (b + 1)],
                                 lhsT=etiles[c][:, 128 * b:128 * (b + 1)],
                                 rhs=u16[:, c, :],
                                 start=(c == 0 and b == 0),
                                 stop=(c == NCH - 1 and b == NCH - 1))
        return psy

    def emit_finA(s, psy):
        psy3 = psy[:, 0:NCH * UD].rearrange("p (c u) -> p c u", u=UD)
        # small early eviction of the D and q columns: the fin/prep chain
        # depends only on these 3 of the 23 columns
        ynq = small.tile([128, NCH, 3], f32, tag="ynq")
        nc.scalar.copy(ynq, psy3[:, :, Din:Din + 3])
        yn = ypool.tile([128, NCH, UD], f32, tag="yn")
        nc.scalar.copy(yn, psy3)
        return yn, ynq

    def emit_finB_head(s, L, yn, ynq=None):
        """1/D + early score-column update off the small early D/q eviction;
        the full-U update uses a PE mask-matmul broadcast of 1/D (emitted in
        emit_finB_tail)."""
        rd = small.tile([128, NCH], f32, tag="rd")
        nc.vector.reciprocal(rd, ynq[:, :, 0] if ynq is not None else yn[:, :, Din])
        psr = psT.tile([NCH, 128], f32, tag="tp", name="psr")
        nc.tensor.transpose(psr, rd, identf)
        rd16 = small.tile([NCH, 128], f16, tag="rd16")
        nc.scalar.copy(rd16, psr)
        psb = psT.tile([128, NCH * UD], f32, tag="tp", name="psb")
        # cmask carries the 2^L denominator rescale (cmask16[L])
        nc.tensor.matmul(psb, lhsT=rd16, rhs=cmask16[:, L, :], start=True, stop=True)
        rdbc = rbpool.tile([128, NCH * UD], f32, tag="rdbc")
        nc.scalar.copy(rdbc, psb)
        snew = None
        if L == 0:   # scores only feed a next layer
            rdbc3 = rdbc.rearrange("p (c u) -> p c u", u=UD)
            qsrc = ynq[:, :, 1:3] if ynq is not None else yn[:, :, Din + 1:Din + 3]
            snew = small.tile([128, NCH, 2], f32, tag="snew")
            tms = small.tile([128, NCH, 2], f32, tag="tms")
            nc.gpsimd.tensor_tensor(tms, qsrc, rdbc3[:, :, Din + 1:Din + 3], Alu.mult)
            nc.gpsimd.tensor_tensor(snew, tms, un[s][:, :, Din + 1:Din + 3], Alu.add)
        return snew, yn, rdbc

    def emit_finB_tail(s, fin, eng=None):
        """Deferred full-U update: only needed by the NEXT sweep's matmuls."""
        eng = eng or nc.gpsimd
        snew, yn, rdbc = fin
        ynf = yn.rearrange("p c u -> p (c u)")
        unf = un[s].rearrange("p c u -> p (c u)")
        tm = rbpool.tile([128, NCH * UD], f32, tag="tm")
        eng.tensor_tensor(tm, ynf, rdbc, Alu.mult)
        new_un = upool.tile([128, NCH, UD], f32, tag="un")
        eng.tensor_tensor(new_un.rearrange("p c u -> p (c u)"), tm, unf, Alu.add)
        new_un16 = u16pool.tile([128, NCH, UD], f16, tag="u16")
        nc.scalar.copy(new_un16, new_un)
        un[s], un16[s] = new_un, new_un16

    def emit_tail(s):
        """hidden = U[:, :21] @ V: one packed [92,128] transpose per 4 chunks;
        per-chunk extraction via 4 zero-padded V tensors (zero rows are free
        in the contraction). All PE ops at tile position (0,0)."""
        u16f = un16[s].rearrange("p c u -> p (c u)")
        for g in range(4):
            pst = psT.tile([92, 128], f16, tag="tp", name=f"pst{g}")
            nc.tensor.transpose(pst, u16f[:, 4 * UD * g:4 * UD * (g + 1)], ident16)
            u2t = u2tp.tile([92, 128], f16, tag="u2t")
            nc.scalar.copy(u2t, pst)
            psh = psH.tile([128, 4, H], f32, tag="psh")
            for j in range(4):
                nc.tensor.matmul(psh[:, j, :], lhsT=u2t, rhs=v92s[:, j, :],
                                 start=True, stop=True)
            hout = houtp.tile([128, 4, H], f32, tag="hout")
            nc.scalar.copy(hout, psh)
            nc.sync.dma_start(
                out=out_ap[s].rearrange("(p c) h -> p c h", c=NCH)[:, 4 * g:4 * g + 4, :],
                in_=hout)

    def emit_last_phase(s, mid=None):
        """Final layer of the last sample: E tiles produced per 4-block group
        so eviction, U-update, and the hidden=U@V tail pipeline with E
        production instead of serializing after the sweep."""
        gbc, pf, ppf = prep[s]
        u16, u = un16[s], un[s]
        new_un = upool.tile([128, NCH, UD], f32, tag="un", name="un_last")
        new_un16 = u16pool.tile([128, NCH, UD], f16, tag="u16", name="u16_last")
        nu16f = new_un16.rearrange("p c u -> p (c u)")
        psy = psY.tile([128, 512], f32, tag="psy", name="psy_last")
        yns = {}

        psy3 = psy[:, 0:NCH * UD].rearrange("p (c u) -> p c u", u=UD)

        def group_fin(g):
            ctx2 = tc.high_priority()
            ctx2.__enter__()
            yn_g = yns[g]
            dsc = small.tile([128, 4], f32, tag="dscg", name=f"dsc{g}")
            nc.vector.tensor_scalar(dsc, yn_g[:, :, Din], 0.5, None, Alu.mult)
            rd = small.tile([128, 4], f32, tag="rdg", name=f"rd{g}")
            nc.vector.reciprocal(rd, dsc)
            for j in range(4):
                cc = 4 * g + j
                nc.vector.scalar_tensor_tensor(new_un[:, cc, :], yn_g[:, j, :],
                                               rd[:, j:j + 1], u[:, cc, :],
                                               Alu.mult, Alu.add)
            (nc.vector.tensor_copy if g >= 2 else nc.scalar.copy)(
                new_un16[:, 4 * g:4 * (g + 1), :],
                new_un[:, 4 * g:4 * (g + 1), :])
            cp = nc.vector.tensor_copy if g >= 2 else nc.scalar.copy
            pst = psT.tile([92, 128], f16, tag="tp", name=f"lpst{g}")
            nc.tensor.transpose(pst, nu16f[:, 92 * g:92 * (g + 1)], ident16)
            u2t = u2tp.tile([92, 128], f16, tag="u2t")
            cp(u2t, pst)
            psh = psH.tile([128, 4, H], f32, tag="psh")
            for j in range(4):
                nc.tensor.matmul(psh[:, j, :], lhsT=u2t, rhs=v92s[:, j, :],
                                 start=True, stop=True)
            hout = houtp.tile([128, 4, H], f32, tag="hout")
            cp(hout, psh)
            nc.sync.dma_start(
                out=out_ap[s].rearrange("(p c) h -> p c h", c=NCH)[:, 4 * g:4 * g + 4, :],
                in_=hout)
            ctx2.__exit__(None, None, None)

        for h in range(2):
            tiles = {}
            order = [NCH - 2, NCH - 1] + list(range(NCH - 2))
            for c in order:
                eng = nc.gpsimd if c >= NCH - 2 else nc.vector
                e_t = e4pool.tile([128, 1024], f16, tag="e4", name=f"e4_{c}")
                eng.tensor_scalar(e_t, gbc[:, 1024 * h:1024 * (h + 1)],
                                  ppf(c), pf(c), Alu.mult, Alu.max)
                tiles[c] = e_t
                if h == 0 and c == 1 and mid is not None:
                    mid()
                if h == 1 and c == 3:
                    group_fin(0)
                    group_fin(1)
            for c in range(NCH):
                for j in range(8):
                    b = 8 * h + j
                    nc.tensor.matmul(psy[:, UD * b:UD * (b + 1)],
                                     lhsT=tiles[c][:, 128 * j:128 * (j + 1)],
                                     rhs=u16[:, c, :],
                                     start=(c == 0 and j == 0),
                                     stop=(c == NCH - 1 and j == 7))
            for gg in range(2):
                g = 2 * h + gg
                yn_g = ypool.tile([128, 4, UD], f32, tag="yng", name=f"yn_g{g}")
                cpy = nc.vector.tensor_copy if h == 1 else nc.scalar.copy
                cpy(yn_g, psy[:, UD * 4 * g:UD * 4 * (g + 1)]
                    .rearrange("p (c u) -> p c u", u=UD))
                yns[g] = yn_g
            if h == 1:
                group_fin(2)
                group_fin(3)

    # ---- schedule: samples interleaved; each fin+prep emitted so it runs
    # during the OTHER sample's sweep; PE/Pool items for fin/prep precede the
    # next sweep's matmul batch in their queues ----
    e00 = emit_sweep_E(0, 0)
    y00 = emit_sweep_MM(0, e00)
    yn00, ynq00 = emit_finA(0, y00)
    fin_state = {}

    def trans_head(s, L, yn, ynq):
        fin = emit_finB_head(s, L, yn, ynq)
        emit_prep(s, fin[0])
        fin_state[s] = fin

    def trans_tail(s):
        emit_finB_tail(s, fin_state.pop(s))
    e10 = emit_sweep_E(1, 0, mid=lambda: trans_head(0, 0, yn00, ynq00),
                       mid2=lambda: trans_tail(0))
    y10 = emit_sweep_MM(1, e10)
    yn10, ynq10 = emit_finA(1, y10)
    e01 = emit_sweep_E(0, 1, mid=lambda: trans_head(1, 0, yn10, ynq10),
                       mid2=lambda: trans_tail(1))
    y01 = emit_sweep_MM(0, e01)
    yn01, ynq01 = emit_finA(0, y01)
    def fin_and_tail0():
        fin = emit_finB_head(0, 1, yn01, ynq01)
        emit_finB_tail(0, fin, eng=nc.vector)
        emit_tail(0)
    emit_last_phase(1, mid=fin_and_tail0)


def _host_prep(inputs):
    x = np.ascontiguousarray(np.asarray(inputs["x"], dtype=np.float32))
    W_in = np.asarray(inputs["W_in"], dtype=np.float32)
    b_in = np.asarray(inputs["b_in"], dtype=np.float32)
    W_t = np.asarray(inputs["W_t"], dtype=np.float32)
    b_t = np.asarray(inputs["b_t"], dtype=np.float32)
    a = np.asarray(inputs["a"], dtype=np.float32)
    a_j, a_i = a[:H, 0], a[H:, 0]
    wj = (W_t @ a_j).astype(np.float32)
    wi = (W_t @ a_i).astype(np.float32)
    V = np.concatenate([W_in, b_in[None, :]], axis=0)          # [21, 128]
    w21 = np.stack([V @ wj, V @ wi], axis=1).astype(np.float32)  # [21, 2]
    ctot = float(np.float32(b_t @ a_j) + np.float32(b_t @ a_i))
    B = x.shape[0]
    xr = x.reshape(B, 128, NCH, Din)                  # node n = 16p + c
    s0 = (xr @ w21[:Din] + w21[Din]).astype(np.float32)  # [B, 128, 16, 2]
    u0f = np.concatenate(
        [xr, np.ones((B, 128, NCH, 1), np.float32), s0], axis=3)  # [B,128,16,23]
    u0f = np.ascontiguousarray(u0f)
    u016 = np.ascontiguousarray(u0f.astype(np.float16))
    s0j, s0i = s0[..., 0], s0[..., 1]
    mx = s0j.max(axis=(1, 2), keepdims=True)
    p0 = np.stack([np.exp(s0j - mx), np.exp(0.01 * (s0j - mx))], axis=3)
    p0 = np.ascontiguousarray(p0.astype(np.float32))           # [B,128,16,2]
    g0 = np.exp(np.minimum(-0.99 * (s0i + mx + np.float32(ctot)), 10.5))
    g0row = g0.transpose(0, 2, 1).reshape(B, N)                # i = 128c + p
    g0bc = np.ascontiguousarray(
        np.broadcast_to(g0row[:, None, :], (B, 128, N)).astype(np.float16))
    sel = np.zeros((NCH, NCH, 128), np.float16)
    for c in range(NCH):
        sel[c, c, :] = 1.0
    identf = np.eye(128, dtype=np.float32)
    ident16 = np.eye(128, dtype=np.float16)
    v92s = np.zeros((92, 4, 128), np.float16)
    for k in range(4):
        v92s[UD * k:UD * k + Din + 1, k, :] = V.astype(np.float16)
    cmask = np.zeros((NCH, 2, NCH * UD), np.float16)
    for L in range(2):
        for c in range(NCH):
            cmask[c, L, UD * c:UD * (c + 1)] = float(2.0 ** L)
    feeds = dict(g0bc=g0bc, p0in=p0, u016in=u016, u0fin=u0f)
    consts = dict(sel16=sel, identf=identf, ident16=ident16, v32=v92s,
                  cmask16=cmask)
    return feeds, consts, ctot


def build_program(ctot):
    import concourse.tile as tile
    from concourse import mybir
    from concourse.bacc import Bacc

    f32 = mybir.dt.float32
    f16 = mybir.dt.float16
    nc = Bacc("TRN2", target_bir_lowering=False, debug=False)
    g0bc_t = nc.dram_tensor("g0bc", [S, 128, N], f16, kind="ExternalInput")
    p0_t = nc.dram_tensor("p0in", [S, 128, NCH, 2], f32, kind="ExternalInput")
    u016_t = nc.dram_tensor("u016in", [S, 128, NCH, UD], f16, kind="ExternalInput")
    u0f_t = nc.dram_tensor("u0fin", [S, 128, NCH, UD], f32, kind="ExternalInput")
    sel_t = nc.dram_tensor("sel16", [NCH, NCH, 128], f16, kind="ExternalInput")
    identf_t = nc.dram_tensor("identf", [128, 128], f32, kind="ExternalInput")
    ident16_t = nc.dram_tensor("ident16", [128, 128], f16, kind="ExternalInput")
    v32_t = nc.dram_tensor("v32", [92, 4, 128], f16, kind="ExternalInput")
    cmask_t = nc.dram_tensor("cmask16", [NCH, 2, NCH * UD], f16, kind="ExternalInput")
    out_t = nc.dram_tensor("out", [S, N, H], f32, kind="ExternalOutput")
    aps = (g0bc_t.ap(), p0_t.ap(), u016_t.ap(), u0f_t.ap(), sel_t.ap(),
           identf_t.ap(), ident16_t.ap(), v32_t.ap(), cmask_t.ap(), out_t.ap())
    with tile.TileContext(nc) as tc, ExitStack() as ctx:
        _build(ctx, tc, aps, ctot)
    nc.compile()
    return nc


def kernel(**inputs) -> np.ndarray:
    from concourse.bass_utils import run_bass_kernel_spmd

    feeds, consts, ctot = _host_prep(inputs)
    B = feeds["g0bc"].shape[0]
    nc = build_program(ctot)
    in_maps = []
    for i in range(N_CORES):
        m = {k: np.ascontiguousarray(v[i * S:(i + 1) * S]) for k, v in feeds.items()}
        m.update(consts)
        in_maps.append(m)
    res = run_bass_kernel_spmd(nc, in_maps, list(range(N_CORES)))
    out = np.concatenate([res.results[i]["out"] for i in range(N_CORES)], axis=0)
    assert out.shape == (B, N, H)
    return out
